# revision 5
# baseline (speedup 1.0000x reference)
"""Trainium2 Bass kernel for nn_AttnDecoderRnn (attention decoder + 2-layer GRU).

Self-contained: kernel(**inputs) -> np.ndarray [32, 512, 512] float32 log-probs.

Sharding: data-parallel over batch across 8 cores (4 examples/core).

v2 GRU recurrence redesign (the dominant phase):
  - PSUM is seeded per step with the precomputed input projections (r/z
    gates) and the hidden n-bias via identity matmuls, eliminating the
    DVE adds from the gate chain.
  - The hidden state lives as two bf16 half-tiles; the 8-chunk weight
    contraction is emitted in (output-half, input-half) phases so the
    tensor engine can start next step's matmuls while the second half of
    the gate chain is still in flight.
  - Gate chain spread across ACT (sigmoid/tanh, incl. the 1-z trick via
    sigmoid(-x)), DVE (muls/adds), and GpSimd (z*h and y copies).
"""
import numpy as np
import ml_dtypes

import concourse.bass as bass
import concourse.bacc as bacc
import concourse.tile as tile
from concourse import mybir
from concourse.bass_utils import run_bass_kernel_spmd

B, T_ENC, H, L, V = 32, 1024, 1024, 511, 512
T = L + 1
SOS = 0
N_CORES = 8
E = B // N_CORES   # 4
NH = 8
NM = 24
HB = NH * E        # 32
F32 = mybir.dt.float32
F32R = mybir.dt.float32r
BF16 = mybir.dt.bfloat16
AF = mybir.ActivationFunctionType
ALU = mybir.AluOpType

_CACHE = {}


def _build():
    nc = bacc.Bacc("TRN2", target_bir_lowering=False, debug=False,
                   num_devices=N_CORES)

    dp = nc.declare_dram_parameter
    enc4 = dp("enc4", [E, NH, 128, H], F32R, isOutput=False)     # s-chunked
    onehot = dp("onehot", [E, 4, 128, T], F32R, isOutput=False)
    maskr = dp("maskr", [E, 128, T_ENC], F32, isOutput=False)
    embT = dp("embT", [128, NH, V], F32R, isOutput=False)
    lininT = dp("lininT", [128, NH, H], F32R, isOutput=False)
    linoutT = dp("linoutT", [128, 16, H], BF16, isOutput=False)
    wih0 = dp("wih0", [128, NH, 3 * H], BF16, isOutput=False)
    wih1 = dp("wih1", [128, NH, 3 * H], BF16, isOutput=False)
    whh0 = dp("whh0", [128, NM * NH * 128], BF16, isOutput=False)
    whh1 = dp("whh1", [128, NM * NH * 128], BF16, isOutput=False)
    biasg0 = dp("biasg0", [128, NM], F32, isOutput=False)
    biasg1 = dp("biasg1", [128, NM], F32, isOutput=False)
    bhn0 = dp("bhn0", [128, HB], BF16, isOutput=False)
    bhn1 = dp("bhn1", [128, HB], BF16, isOutput=False)
    fcwT = dp("fcwT", [128, NH, V], BF16, isOutput=False)
    fcbr = dp("fcbr", [128, V], F32, isOutput=False)
    ident = dp("ident", [128, 128], F32R, isOutput=False)
    identg = dp("identg", [128, 128], BF16, isOutput=False)
    out = dp("out", [E, T, V], F32, isOutput=True)

    xp0 = nc.dram_tensor("xp0", [NM, E, 128, T], BF16)
    xp1 = nc.dram_tensor("xp1", [NM, E, 128, T], BF16)
    qtabd = nc.dram_tensor("qtabd", [4, 128, H], F32)            # Q_table [v, h]
    attnTd = nc.dram_tensor("attnTd", [NH, 128, E, T], BF16)     # attn_out.T
    y0T = nc.dram_tensor("y0T", [NH, 128, E, T], BF16)
    y1T = nc.dram_tensor("y1T", [NH, 128, E, T], BF16)

    with tile.TileContext(nc) as tc:
        # ---------- Q_table = emb @ lin_in.T : [v, h] -> DRAM ----------
        with (
            tc.tile_pool(name="qa", bufs=1) as qa,
            tc.tile_pool(name="qps", bufs=2, space="PSUM") as qps,
        ):
            embT_sb = qa.tile([128, NH, V], F32R)
            nc.sync.dma_start(embT_sb[:], embT[:, :, :])
            lininT_sb = qa.tile([128, NH, H], F32R)
            nc.sync.dma_start(lininT_sb[:], lininT[:, :, :])
            for vm in range(4):
                for half in range(2):
                    psq = qps.tile([128, 512], F32, tag="psq")
                    for k in range(NH):
                        nc.tensor.matmul(
                            psq[:],
                            embT_sb[:, k, vm * 128:(vm + 1) * 128],
                            lininT_sb[:, k, half * 512:(half + 1) * 512],
                            start=(k == 0), stop=(k == NH - 1))
                    qcp = qa.tile([128, 512], F32, tag="qcp")
                    nc.vector.tensor_copy(qcp[:], psq[:])
                    nc.sync.dma_start(
                        qtabd[vm, :, half * 512:(half + 1) * 512], qcp[:])

        # ---------- Phase A: attention per example ----------
        with (
            tc.tile_pool(name="apers", bufs=1) as apers,
            tc.tile_pool(name="pex", bufs=1) as pex,
            tc.tile_pool(name="encs", bufs=3) as encs,
            tc.tile_pool(name="sm", bufs=2) as sm,
            tc.tile_pool(name="pp", bufs=2, space="PSUM") as pp,
            tc.tile_pool(name="ppt", bufs=2, space="PSUM") as ppt,
            tc.tile_pool(name="pps", bufs=1, space="PSUM") as pps,
        ):
            id_sb = apers.tile([128, 128], F32R)
            nc.sync.dma_start(id_sb[:], ident[:, :])
            linoutT_sb = apers.tile([128, 16, H], BF16)
            nc.sync.dma_start(linoutT_sb[:], linoutT[:, :, :])
            q_sb = apers.tile([128, 4, H], F32R)
            nc.gpsimd.dma_start(q_sb[:], qtabd.rearrange("v p h -> p v h"))

            for e in range(E):
                # q.T [h, t] via one-hot matmul
                oh = pex.tile([128, 4, T], F32R, tag="oh")
                nc.sync.dma_start(oh[:], onehot[e].rearrange("v p t -> p v t"))
                qT = pex.tile([128, NH, T], F32R, tag="qT")
                for hm in range(NH):
                    psn = pp.tile([128, T], F32, tag="pse")
                    for k in range(4):
                        nc.tensor.matmul(
                            psn[:],
                            q_sb[:, k, hm * 128:(hm + 1) * 128],
                            oh[:, k, :],
                            start=(k == 0), stop=(k == 3))
                    nc.vector.tensor_copy(qT[:, hm, :], psn[:])

                # enc.T [h, s] via PE transposes (enc chunks streamed)
                encT = pex.tile([128, NH, T_ENC], F32R, tag="encT")
                for sc in range(NH):
                    ecn = encs.tile([128, H], F32R, tag="ecn")
                    nc.sync.dma_start(ecn[:], enc4[e, sc])
                    for hg in range(2):
                        pst_ = ppt.tile([128, 512], F32R, tag="ptr")
                        for hi in range(4):
                            hc = hg * 4 + hi
                            nc.tensor.transpose(
                                pst_[:, hi * 128:(hi + 1) * 128],
                                ecn[:, hc * 128:(hc + 1) * 128],
                                id_sb[:])
                        for hi in range(4):
                            hc = hg * 4 + hi
                            nc.vector.tensor_copy(
                                encT[:, hc, sc * 128:(sc + 1) * 128],
                                pst_[:, hi * 128:(hi + 1) * 128])

                mk = pex.tile([128, T_ENC], F32, tag="mk")
                nc.sync.dma_start(mk[:], maskr[e])

                # scores + softmax + w.T
                wT = pex.tile([128, NH, T], F32R, tag="wT")
                for tm in range(4):
                    sc_ps = pps.tile([128, T_ENC], F32, tag="scps")
                    for half in range(2):
                        for k in range(NH):
                            nc.tensor.matmul(
                                sc_ps[:, half * 512:(half + 1) * 512],
                                qT[:, k, tm * 128:(tm + 1) * 128],
                                encT[:, k, half * 512:(half + 1) * 512],
                                start=(k == 0), stop=(k == NH - 1))
                    scs = sm.tile([128, T_ENC], F32, tag="scs")
                    nc.vector.tensor_add(scs[:], sc_ps[:], mk[:])
                    mx = sm.tile([128, 1], F32, tag="mx")
                    nc.vector.reduce_max(mx[:], scs[:], axis=mybir.AxisListType.X)
                    nmx = sm.tile([128, 1], F32, tag="nmx")
                    nc.vector.tensor_scalar_mul(nmx[:], mx[:], -1.0)
                    ex_t = sm.tile([128, T_ENC], F32, tag="ex")
                    nc.scalar.activation(ex_t[:], scs[:], AF.Exp, bias=nmx[:])
                    sme = sm.tile([128, 1], F32, tag="sme")
                    nc.vector.reduce_sum(sme[:], ex_t[:], axis=mybir.AxisListType.X)
                    rc = sm.tile([128, 1], F32, tag="rc")
                    nc.vector.reciprocal(rc[:], sme[:])
                    wsm = sm.tile([128, T_ENC], F32R, tag="wsm")
                    nc.vector.tensor_scalar_mul(wsm[:], ex_t[:], rc[:])
                    for sg in range(2):
                        pst2 = ppt.tile([128, 512], F32R, tag="ptr")
                        for si in range(4):
                            nc.tensor.transpose(
                                pst2[:, si * 128:(si + 1) * 128],
                                wsm[:, (sg * 4 + si) * 128:(sg * 4 + si + 1) * 128],
                                id_sb[:])
                        for si in range(4):
                            nc.vector.tensor_copy(
                                wT[:, sg * 4 + si, tm * 128:(tm + 1) * 128],
                                pst2[:, si * 128:(si + 1) * 128])

                # mix.T [d, t]: lhsT = enc chunks (streamed again), rhs = wT
                combT = pex.tile([128, 16, T], BF16, tag="combT")
                for hm in range(NH):
                    nc.vector.tensor_copy(combT[:, 8 + hm, :], qT[:, hm, :])
                for dm in range(NH):
                    psm = pp.tile([128, T], F32, tag="pse")
                    for k in range(NH):
                        eck = encs.tile([128, 128], F32R, tag="eck")
                        nc.sync.dma_start(
                            eck[:], enc4[e, k, :, dm * 128:(dm + 1) * 128])
                        nc.tensor.matmul(
                            psm[:], eck[:], wT[:, k, :],
                            start=(k == 0), stop=(k == NH - 1))
                    nc.vector.tensor_copy(combT[:, dm, :], psm[:])

                # attn_out.T = tanh(lin_out.T-contract @ combined.T) -> DRAM
                for dm in range(NH):
                    psa = pp.tile([128, T], F32, tag="pse")
                    for c in range(16):
                        nc.tensor.matmul(
                            psa[:],
                            linoutT_sb[:, c, dm * 128:(dm + 1) * 128],
                            combT[:, c, :],
                            start=(c == 0), stop=(c == 15))
                    att = sm.tile([128, T], BF16, tag="att")
                    nc.scalar.activation(att[:], psa[:], AF.Tanh)
                    nc.sync.dma_start(attnTd[dm, :, e, :], att[:])

        # ---------- Phase A2 / Bm: xp = src @ w_ih.T + biases ----------
        def xp_phase(wih_dram, src_dram, biasg_dram, xp_dram):
            with (
                tc.tile_pool(name="xw", bufs=1) as xw,
                tc.tile_pool(name="xs", bufs=2) as xs,
                tc.tile_pool(name="xpp", bufs=2, space="PSUM") as xpp,
            ):
                wih_sb = xw.tile([128, NH, 3 * H], BF16, name="wih_sb")
                nc.sync.dma_start(wih_sb[:], wih_dram[:, :, :])
                src_sb = xw.tile([128, NH, E * T], BF16, name="src_sb")
                nc.sync.dma_start(
                    src_sb[:], src_dram.rearrange("c p e t -> p c (e t)"))
                bg = xw.tile([128, NM], F32, name="bg")
                nc.sync.dma_start(bg[:], biasg_dram[:, :])
                for m in range(NM):
                    for e in range(E):
                        psx = xpp.tile([128, T], F32, tag="psx")
                        for k in range(NH):
                            nc.tensor.matmul(
                                psx[:],
                                wih_sb[:, k, m * 128:(m + 1) * 128],
                                src_sb[:, k, e * T:(e + 1) * T],
                                start=(k == 0), stop=(k == NH - 1))
                        xps = xs.tile([128, T], BF16, tag="xps")
                        nc.vector.tensor_scalar_add(xps[:], psx[:], bg[:, m:m + 1])
                        nc.sync.dma_start(xp_dram[m, e], xps[:])

        xp_phase(wih0, attnTd, biasg0, xp0)

        # ---------- Phase B: GRU layers (v2 step) ----------
        def gru_layer(xp_dram, whh_dram, bhn_dram, yT_dram, tblk):
            with (
                tc.tile_pool(name="gw", bufs=1) as gw,
                tc.tile_pool(name="gx", bufs=2) as gx,
                tc.tile_pool(name="gh", bufs=1) as gh,
                tc.tile_pool(name="gg", bufs=4) as gg,
                tc.tile_pool(name="gy", bufs=2) as gy,
                tc.tile_pool(name="gps", bufs=2, space="PSUM") as gps,
            ):
                whh_sb = gw.tile([128, NM * NH * 128], BF16, name="whh_sb")
                nc.sync.dma_start(whh_sb[:], whh_dram[:, :])
                bhn_sb = gw.tile([128, HB], BF16, name="bhn_sb")
                nc.sync.dma_start(bhn_sb[:], bhn_dram[:, :])
                idg_sb = gw.tile([128, 128], BF16, name="idg_sb")
                nc.sync.dma_start(idg_sb[:], identg[:, :])
                # bf16 hidden state: [parity][output-half] -> [128, 16]
                hbh = [[gh.tile([128, HB // 2], BF16, name=f"hb{p}{u}",
                                tag=f"hb{p}{u}") for u in range(2)]
                       for p in range(2)]
                for u in range(2):
                    nc.vector.memset(hbh[0][u][:], 0.0)

                xpr = xp_dram.rearrange("m e p t -> p m e t")
                r3h = lambda ap: ap.rearrange("p (c e) -> p c e", c=4)

                def step(xpt, xsd, ti, cur, nxt, yblk):
                    # per-half PSUM tiles (separate banks: PE writes to one
                    # half's banks never collide with chain reads of the other)
                    ps_rz = [gps.tile([128, 32], F32, tag=f"psrz{u}",
                                      name=f"psrz{u}_{ti}") for u in range(2)]
                    ps_n = [gps.tile([128, 16], F32, tag=f"psn{u}",
                                     name=f"psn{u}_{ti}") for u in range(2)]
                    # seeds: ps_rz[u] <- [xr_u | xz_u] ; ps_n[u] <- b_hn
                    for u in range(2):
                        nc.tensor.matmul(ps_rz[u][:], idg_sb[:],
                                         xsd[:, ti, u],
                                         start=True, stop=False,
                                         skip_group_check=True)
                        nc.tensor.matmul(ps_n[u][:], idg_sb[:],
                                         bhn_sb[:, u * 16:(u + 1) * 16],
                                         start=True, stop=False,
                                         skip_group_check=True)
                    # weight contraction, phases (output-half u, input-half
                    # jh). Order U0J0,U0J1,U1J1,U1J0: U0's psum completes as
                    # early as possible; U1 finishes on the old (J0) h half so
                    # it never waits on the late half of the previous chain.
                    last_phase = {0: 1, 1: 0}
                    for (u, jh) in ((0, 0), (0, 1), (1, 1), (1, 0)):
                        for g in range(3):
                            for ci in range(4):
                                c = u * 4 + ci
                                m = g * NH + c
                                if g < 2:
                                    pst, colb = ps_rz[u], g * 16 + ci * 4
                                else:
                                    pst, colb = ps_n[u], ci * 4
                                for jj in range(4):
                                    j = jh * 4 + jj
                                    nc.tensor.matmul(
                                        pst[:, colb:colb + 4],
                                        whh_sb[:, (m * NH + j) * 128:
                                               (m * NH + j + 1) * 128],
                                        hbh[cur][j // 4][:, (j % 4) * 4:
                                                         (j % 4 + 1) * 4],
                                        start=False,
                                        stop=(jh == last_phase[u] and jj == 3),
                                        skip_group_check=True)
                    # gate chains (two halves, interleaved emission)
                    r_ = [None, None]
                    z_ = [None, None]
                    z1m_ = [None, None]
                    t2_ = [None, None]
                    n_ = [None, None]
                    zh_ = [None, None]
                    for u in range(2):
                        r_[u] = gg.tile([128, 16], F32, tag=f"r{u}",
                                        name=f"r{u}_{ti}")
                        nc.scalar.activation(r_[u][:], ps_rz[u][:, 0:16],
                                             AF.Sigmoid)
                        z_[u] = gg.tile([128, 16], F32, tag=f"z{u}",
                                        name=f"z{u}_{ti}")
                        nc.scalar.activation(z_[u][:], ps_rz[u][:, 16:32],
                                             AF.Sigmoid)
                        z1m_[u] = gg.tile([128, 16], F32, tag=f"zm{u}",
                                          name=f"zm{u}_{ti}")
                        nc.scalar.activation(z1m_[u][:], ps_rz[u][:, 16:32],
                                             AF.Sigmoid, scale=-1.0)
                    for u in range(2):
                        t1 = gg.tile([128, 16], F32, tag=f"t1{u}",
                                     name=f"t1{u}_{ti}")
                        nc.vector.tensor_mul(t1[:], r_[u][:], ps_n[u][:])
                        t2_[u] = gg.tile([128, 16], F32, tag=f"t2{u}",
                                         name=f"t2{u}_{ti}")
                        nc.vector.tensor_add(
                            r3h(t2_[u][:]), r3h(t1[:]),
                            xpt[:, 16 + u * 4:16 + u * 4 + 4, :, ti])
                        zh_[u] = gg.tile([128, 16], F32, tag=f"zh{u}",
                                         name=f"zh{u}_{ti}")
                        nc.gpsimd.tensor_mul(zh_[u][:], z_[u][:],
                                             hbh[cur][u][:])
                    for u in range(2):
                        n_[u] = gg.tile([128, 16], F32, tag=f"n{u}",
                                        name=f"n{u}_{ti}")
                        nc.scalar.activation(n_[u][:], t2_[u][:], AF.Tanh)
                    for u in range(2):
                        t3 = gg.tile([128, 16], F32, tag=f"t3{u}",
                                     name=f"t3{u}_{ti}")
                        nc.vector.tensor_mul(t3[:], n_[u][:], z1m_[u][:])
                        nc.gpsimd.tensor_add(hbh[nxt][u][:], t3[:],
                                             zh_[u][:])
                    for u in range(2):
                        nc.gpsimd.tensor_copy(
                            yblk[:, u * 4:(u + 1) * 4, :, ti],
                            r3h(hbh[nxt][u][:]))

                for tb in range(T // tblk):
                    xpt = gx.tile([128, NM, E, tblk], BF16, tag="xpt")
                    nc.sync.dma_start(
                        xpt[:], xpr[:, :, :, tb * tblk:(tb + 1) * tblk])
                    # contiguous per-step seed operand: [t, u, (g c e)]
                    xsd = gx.tile([128, tblk, 2, 2, 4, 4], BF16, tag="xsd")
                    nc.vector.tensor_copy(
                        xsd[:],
                        xpt[:, 0:16, :, :].rearrange(
                            "p (g uu c) e t -> p t uu g c e", g=2, uu=2))
                    yblk = gy.tile([128, NH, E, tblk], BF16, tag="yblk")
                    for ti in range(tblk):
                        t = tb * tblk + ti
                        step(xpt, xsd, ti, t % 2, (t + 1) % 2, yblk)
                    for cc in range(NH):
                        nc.sync.dma_start(
                            yT_dram[cc, :, :, tb * tblk:(tb + 1) * tblk],
                            yblk[:, cc])

        gru_layer(xp0, whh0, bhn0, y0T, 128)
        xp_phase(wih1, y0T, biasg1, xp1)
        gru_layer(xp1, whh1, bhn1, y1T, 128)

        # ---------- Phase C ----------
        with (
            tc.tile_pool(name="cw", bufs=1) as cw,
            tc.tile_pool(name="cs", bufs=2) as cs,
            tc.tile_pool(name="cpp", bufs=2, space="PSUM") as cpp,
        ):
            fcw_sb = cw.tile([128, NH, V], BF16)
            nc.sync.dma_start(fcw_sb[:], fcwT[:, :, :])
            fcb_sb = cw.tile([128, V], F32)
            nc.sync.dma_start(fcb_sb[:], fcbr[:, :])
            y1_sb = cw.tile([128, NH, E * T], BF16)
            nc.gpsimd.dma_start(y1_sb[:], y1T.rearrange("c p e t -> p c (e t)"))
            for e in range(E):
                for tm in range(4):
                    pl = cpp.tile([128, V], F32, tag="pl")
                    for k in range(NH):
                        nc.tensor.matmul(
                            pl[:],
                            y1_sb[:, k, e * T + tm * 128: e * T + (tm + 1) * 128],
                            fcw_sb[:, k, :],
                            start=(k == 0), stop=(k == NH - 1))
                    lg = cs.tile([128, V], F32, tag="lg")
                    nc.vector.tensor_add(lg[:], pl[:], fcb_sb[:])
                    mx = cs.tile([128, 1], F32, tag="cmx")
                    nc.vector.reduce_max(mx[:], lg[:], axis=mybir.AxisListType.X)
                    nmx = cs.tile([128, 1], F32, tag="cnmx")
                    nc.vector.tensor_scalar_mul(nmx[:], mx[:], -1.0)
                    xm = cs.tile([128, V], F32, tag="cxm")
                    nc.vector.tensor_scalar_add(xm[:], lg[:], nmx[:])
                    ext = cs.tile([128, V], F32, tag="cex")
                    nc.scalar.activation(ext[:], lg[:], AF.Exp, bias=nmx[:])
                    sme = cs.tile([128, 1], F32, tag="csm")
                    nc.vector.reduce_sum(sme[:], ext[:], axis=mybir.AxisListType.X)
                    lns = cs.tile([128, 1], F32, tag="clns")
                    nc.scalar.activation(lns[:], sme[:], AF.Ln)
                    nlns = cs.tile([128, 1], F32, tag="cnl")
                    nc.vector.tensor_scalar_mul(nlns[:], lns[:], -1.0)
                    og = cs.tile([128, V], F32, tag="cog")
                    nc.vector.tensor_scalar_add(og[:], xm[:], nlns[:])
                    nc.sync.dma_start(out[e, tm * 128:(tm + 1) * 128, :], og[:])

    nc.compile()
    return nc


def _prep_core(c, inputs):
    enc = np.asarray(inputs["encoder_outputs"], np.float32)
    lens = np.asarray(inputs["encoder_output_lengths"]).astype(np.int64)
    labels = np.asarray(inputs["input_labels"]).astype(np.int64)
    emb = np.asarray(inputs["emb"], np.float32)
    lin_in = np.asarray(inputs["lin_in"], np.float32)
    lin_out = np.asarray(inputs["lin_out"], np.float32)
    fc_w = np.asarray(inputs["fc_w"], np.float32)
    fc_b = np.asarray(inputs["fc_b"], np.float32)

    ex = slice(c * E, (c + 1) * E)
    m = {}
    m["enc4"] = np.ascontiguousarray(
        enc[ex].reshape(E, NH, 128, H))

    lab = np.concatenate(
        [np.full((E, 1), SOS, np.int64), labels[ex]], axis=1)
    oh = np.zeros((E, V, T), np.float32)
    for e in range(E):
        oh[e, lab[e], np.arange(T)] = 1.0
    m["onehot"] = np.ascontiguousarray(oh.reshape(E, 4, 128, T))

    msk = np.zeros((E, T_ENC), np.float32)
    for e in range(E):
        msk[e, lens[c * E + e]:] = -1e30
    m["maskr"] = np.ascontiguousarray(
        np.broadcast_to(msk[:, None, :], (E, 128, T_ENC)))

    def chunks_T(a):
        R, C = a.shape
        return np.ascontiguousarray(a.reshape(R // 128, 128, C).transpose(1, 0, 2))

    m["embT"] = chunks_T(emb.T)
    m["lininT"] = chunks_T(lin_in.T)
    m["linoutT"] = chunks_T(lin_out.T).astype(ml_dtypes.bfloat16)
    m["fcwT"] = chunks_T(fc_w.T).astype(ml_dtypes.bfloat16)
    m["fcbr"] = np.ascontiguousarray(
        np.broadcast_to(fc_b[None, :], (128, V)).astype(np.float32))
    m["ident"] = np.eye(128, dtype=np.float32)
    m["identg"] = np.eye(128, dtype=np.float32).astype(ml_dtypes.bfloat16)

    for li in range(2):
        w_ih = np.asarray(inputs[f"gru_w_ih{li}"], np.float32)
        w_hh = np.asarray(inputs[f"gru_w_hh{li}"], np.float32)
        b_ih = np.asarray(inputs[f"gru_b_ih{li}"], np.float32)
        b_hh = np.asarray(inputs[f"gru_b_hh{li}"], np.float32)
        m[f"wih{li}"] = chunks_T(w_ih.T).astype(ml_dtypes.bfloat16)
        wT = w_hh.T
        pk = np.zeros((128, NM * NH * 128), np.float32)
        for mt in range(NM):
            for j in range(NH):
                pk[:, (mt * NH + j) * 128:(mt * NH + j + 1) * 128] = \
                    wT[j * 128:(j + 1) * 128, mt * 128:(mt + 1) * 128]
        m[f"whh{li}"] = pk.astype(ml_dtypes.bfloat16)
        bg = np.zeros((128, NM), np.float32)
        for g in range(3):
            for hm in range(NH):
                mt = g * NH + hm
                v_ = b_ih[g * H + hm * 128: g * H + (hm + 1) * 128].copy()
                if g < 2:
                    v_ += b_hh[g * H + hm * 128: g * H + (hm + 1) * 128]
                bg[:, mt] = v_
        m[f"biasg{li}"] = bg
        bh = b_hh[2 * H:3 * H].reshape(NH, 128).T
        m[f"bhn{li}"] = np.ascontiguousarray(
            np.repeat(bh[:, :, None], E, axis=2).reshape(128, HB)).astype(
                ml_dtypes.bfloat16)
    return m


def kernel(**inputs) -> np.ndarray:
    if "nc" not in _CACHE:
        _CACHE["nc"] = _build()
    nc = _CACHE["nc"]
    in_maps = [_prep_core(c, inputs) for c in range(N_CORES)]
    res = run_bass_kernel_spmd(nc, in_maps, list(range(N_CORES)), trace=False)
    outp = np.concatenate([res.results[c]["out"] for c in range(N_CORES)], axis=0)
    return outp.astype(np.float32)
